# revision 16
# baseline (speedup 1.0000x reference)
"""FMoE forward (NaiveGate top-2, per-expert FFN, score-weighted combine) on 8 trn2 cores.

Strategy: expert-parallel. The gate (tiny: 4096x1024x8 matmul + top-2 softmax)
is computed on host as part of dispatch; tokens are gathered per expert on
host ("all-to-all by expert id"), each of the 8 cores runs one expert's FFN
(x @ W1 -> gelu -> *gate_w -> @ W2) over its routed tokens, and the host
scatter-adds the per-expert outputs back into the full [4096, 1024] output.
The per-token gate weight commutes with the W2 contraction, so it is folded
into H right after the gelu; padded slots carry weight 0 and contribute
nothing.

Device kernel (per core, capacity C = max expert load, fp16 matmuls, fp32 accum):
  mm1: stationary = W1 chunk [128k, 128h], moving = X^T slab [128k, S tok]
       -> H^T chunk [128h, S] in PSUM (accum over 8 k-chunks); ScalarE
       tanh-gelu (+b1) to fp32, VectorE multiply by broadcast gate weights,
       write H^T to SBUF in fp16.
  mm2: stationary = W2 chunk [128h, 128d], moving = H^T chunk [128h, S tok]
       -> Y^T [128d, S] in PSUM (accum over 32 h-chunks), copy, DMA out.

Both matmuls stream the token dim, so C needs no 128-alignment and the PE
work scales with the true max expert load. All DRAM inputs are host
pre-arranged so every DMA reads contiguous per-partition runs.
"""

import os
import sys

import numpy as np

for _p in ("/opt/trn_rl_repo",):
    if _p not in sys.path and os.path.isdir(_p):
        sys.path.insert(0, _p)

N_TOKENS = 4096
D_MODEL = 1024
D_HIDDEN = 4096
N_EXPERT = 8
TOP_K = 2
P = 128
KO = D_MODEL // P  # 8 contraction chunks for mm1
HO = D_HIDDEN // P  # 32 contraction chunks for mm2
DM = D_MODEL // P  # 8 output-partition chunks of Y^T
NCORES = 8
W1_BLOCKS = 8  # W1 streamed in 8 column blocks of 512
HB = D_HIDDEN // W1_BLOCKS
HO_PER_B = HB // P

# filled by kernel() for test harness introspection
last_results = None

_nc_cache = {}


def _slabs_for(C):
    """Token slabs: first slab 512 so slab-0 mm1 PE time covers the weight
    DMA stream; remainder split evenly (moving dim <= 512 per matmul)."""
    if C <= 512:
        return [C]
    rem = C - 512
    parts = -(-rem // 512)
    base, extra = divmod(rem, parts)
    return [512] + [base + 1] * extra + [base] * (parts - extra)


def _build_bass(C):
    import concourse.mybir as mybir
    from concourse import bacc
    from concourse.tile import TileContext

    f16 = mybir.dt.float16
    f32 = mybir.dt.float32
    GELU = mybir.ActivationFunctionType.Gelu_apprx_tanh

    nc = bacc.Bacc("TRN2", target_bir_lowering=False, debug=False, num_devices=NCORES)

    slabs = _slabs_for(C)
    SMAX = max(slabs)

    x_d = nc.declare_dram_parameter("x", [P, KO * C], f16, isOutput=False)
    w1_d = nc.declare_dram_parameter("w1", [W1_BLOCKS, P, KO, HB], f16, isOutput=False)
    w2_d = nc.declare_dram_parameter("w2", [P, HO, D_MODEL], f16, isOutput=False)
    b1_d = nc.declare_dram_parameter("b1", [P, HO], f32, isOutput=False)
    wg_d = nc.declare_dram_parameter("wg", [1, C], f32, isOutput=False)
    out_d = nc.declare_dram_parameter("out", [DM, P, C], f32, isOutput=True)

    with TileContext(nc) as tc:
        with (
            tc.tile_pool(name="wpool", bufs=1) as wpool,
            tc.tile_pool(name="xpool", bufs=2) as xpool,
            tc.tile_pool(name="hpool", bufs=1) as hpool,
            tc.tile_pool(name="ypool", bufs=2) as ypool,
            tc.tile_pool(name="ps1", bufs=3, space="PSUM") as ps1,
            tc.tile_pool(name="ps2", bufs=3, space="PSUM") as ps2,
        ):
            # PE warm-up: dependency-free matmuls on a scratch tile keep the
            # PE busy during the DMA lead-in so HAM is at full clock when the
            # first real matmul issues.
            warm = wpool.tile([P, 512], f16)
            nc.vector.memset(warm, 0.0)
            wps = ps1.tile([P, 512], mybir.dt.float32, tag="hps")
            for _ in range(26):
                nc.tensor.matmul(
                    wps, lhsT=warm[:, :P], rhs=warm, start=True, stop=True
                )

            # DMA issue order matters. DMAs round-robin over 8 parallel HW
            # queues which share HBM bandwidth, so the first-issued DMAs
            # are exactly the critical set for the first mm1 chain (tiny
            # tensors + X slab 0 + W1 block 0, the big two split 3-ways);
            # queue reuse then serializes the remaining weight stream behind
            # them. Issue the critical set from three engine sequencers in
            # parallel — a single sequencer takes ~0.7us per dma_start just
            # generating descriptors.
            b1_sb = wpool.tile([P, HO], f32)
            nc.scalar.dma_start(b1_sb, b1_d[:, :])
            wg_row = wpool.tile([1, C], f32)
            nc.scalar.dma_start(wg_row, wg_d[:, :])
            wb_sb = wpool.tile([P, C], f32)
            nc.gpsimd.partition_broadcast(wb_sb, wg_row)

            w1_sb = wpool.tile([P, KO, D_HIDDEN], f16)
            w2_sb = wpool.tile([P, HO, D_MODEL], f16)

            tok0 = 0
            for si, S in enumerate(slabs):
                x_sb = xpool.tile([P, KO, SMAX], f16, tag="x", name="x_sb")[:, :, :S]
                x_src = x_d[:, KO * tok0 : KO * (tok0 + S)].rearrange(
                    "p (ko t) -> p ko t", t=S
                )
                if si == 0:
                    for lo, hi in ((0, 3), (3, 6), (6, KO)):
                        nc.sync.dma_start(x_sb[:, lo:hi, :], x_src[:, lo:hi, :])
                        nc.gpsimd.dma_start(
                            w1_sb[:, lo:hi, :HB], w1_d[0][:, lo:hi, :]
                        )
                else:
                    nc.sync.dma_start(x_sb, x_src)
                h_sb = hpool.tile([P, HO, SMAX], f16, tag="h", name="h_sb")[:, :, :S]
                for ho in range(HO):
                    if si == 0 and ho % HO_PER_B == 0 and ho > 0:
                        # stream the remaining W1 blocks in ahead of first use
                        blk = ho // HO_PER_B
                        nc.sync.dma_start(
                            w1_sb[:, :, blk * HB : (blk + 1) * HB], w1_d[blk]
                        )
                    hps = ps1.tile([P, SMAX], mybir.dt.float32, tag="hps", name="hps")[:, :S]
                    for k in range(KO):
                        nc.tensor.matmul(
                            hps,
                            lhsT=w1_sb[:, k, ho * P : (ho + 1) * P],
                            rhs=x_sb[:, k, :],
                            start=(k == 0),
                            stop=(k == KO - 1),
                        )
                    nc.scalar.activation(
                        h_sb[:, ho, :], hps, GELU, bias=b1_sb[:, ho : ho + 1]
                    )
                if si == 0:
                    # W2 behind all of W1 on the queue; needed only for mm2
                    qs = HO // 4
                    for q in range(4):
                        nc.sync.dma_start(
                            w2_sb[:, q * qs : (q + 1) * qs, :],
                            w2_d[:, q * qs : (q + 1) * qs, :],
                        )
                for m in range(DM):
                    yps = ps2.tile([P, SMAX], mybir.dt.float32, tag="yps", name="yps")[:, :S]
                    for ho in range(HO):
                        nc.tensor.matmul(
                            yps,
                            lhsT=w2_sb[:, ho, m * P : (m + 1) * P],
                            rhs=h_sb[:, ho, :],
                            start=(ho == 0),
                            stop=(ho == HO - 1),
                        )
                    y_sb = ypool.tile([P, SMAX], f32, tag="y", name="y_sb")[:, :S]
                    # gate-weight scale on the token (free) dim, fp32 exact
                    nc.vector.tensor_mul(y_sb, yps, wb_sb[:, tok0 : tok0 + S])
                    nc.sync.dma_start(out_d[m, :, tok0 : tok0 + S], y_sb)
                tok0 += S
    nc.compile()
    return nc


def _route(moe_inp, Wg, bg):
    """Host gate: replicates NaiveGate (linear logits, top-2, softmax over the
    selected logits). Returns per-expert (token_idx, combine_weight)."""
    logits = moe_inp.astype(np.float32) @ Wg.astype(np.float32) + bg.astype(np.float32)
    order = np.argsort(-logits, axis=1, kind="stable")  # ties -> lower index first
    top_idx = order[:, :TOP_K]
    top_val = np.take_along_axis(logits, top_idx, axis=1)
    m = top_val.max(axis=1, keepdims=True)
    e = np.exp(top_val - m)
    gate = (e / e.sum(axis=1, keepdims=True)).astype(np.float32)
    toks, weights = [], []
    for ex in range(N_EXPERT):
        mask = top_idx == ex  # [N, K]; each token matches at most one slot
        t = np.nonzero(mask.any(axis=1))[0]
        w = gate[mask]  # row-major -> ascending token order, matches t
        toks.append(t)
        weights.append(w)
    return toks, weights


def kernel(**inputs):
    global last_results
    from concourse.bass_utils import run_bass_kernel_spmd

    moe_inp = np.asarray(inputs["moe_inp"], dtype=np.float32)
    Wg = np.asarray(inputs["Wg"], dtype=np.float32)
    bg = np.asarray(inputs["bg"], dtype=np.float32)
    W1 = np.asarray(inputs["W1"], dtype=np.float32)
    b1 = np.asarray(inputs["b1"], dtype=np.float32)
    W2 = np.asarray(inputs["W2"], dtype=np.float32)
    b2 = np.asarray(inputs["b2"], dtype=np.float32)

    toks, weights = _route(moe_inp, Wg, bg)
    maxc = max(len(t) for t in toks)
    C = max(8, -(-maxc // 4) * 4)  # pad to multiple of 4 for DMA alignment
    slabs = _slabs_for(C)

    if C not in _nc_cache:
        _nc_cache[C] = _build_bass(C)
    nc = _nc_cache[C]

    in_maps = []
    for ex in range(N_EXPERT):
        t, w = toks[ex], weights[ex]
        ce = len(t)
        xT = np.zeros((D_MODEL, C), dtype=np.float16)
        xT[:, :ce] = moe_inp[t].T
        # concatenated partition-major slab blocks [P, KO*C]
        blocks = []
        tok0 = 0
        for S in slabs:
            blocks.append(
                xT[:, tok0 : tok0 + S]
                .reshape(KO, P, S)
                .transpose(1, 0, 2)
                .reshape(P, KO * S)
            )
            tok0 += S
        x_arr = np.ascontiguousarray(np.concatenate(blocks, axis=1))
        w1_f16 = W1[ex].astype(np.float16)  # [1024, 4096]
        w1_arr = np.ascontiguousarray(
            w1_f16.reshape(KO, P, W1_BLOCKS, HB).transpose(2, 1, 0, 3)
        )
        w2_arr = np.ascontiguousarray(
            W2[ex].astype(np.float16).reshape(HO, P, D_MODEL).transpose(1, 0, 2)
        )
        b1_arr = np.ascontiguousarray(b1[ex].reshape(HO, P).T)
        wg_pad = np.zeros((1, C), dtype=np.float32)
        wg_pad[0, :ce] = w
        in_maps.append(
            {"x": x_arr, "w1": w1_arr, "w2": w2_arr, "b1": b1_arr, "wg": wg_pad}
        )

    last_results = run_bass_kernel_spmd(nc, in_maps, core_ids=list(range(NCORES)))

    out = np.zeros((N_TOKENS, D_MODEL), dtype=np.float32)
    for ex in range(N_EXPERT):
        t, w = toks[ex], weights[ex]
        ce = len(t)
        yT = last_results.results[ex]["out"].reshape(D_MODEL, C)  # [1024, C]
        out[t] += yT[:, :ce].T + w[:, None] * b2[ex][None, :]
    return out


if __name__ == "__main__":
    rng = np.random.default_rng(0)
    demo = {
        "moe_inp": rng.standard_normal((N_TOKENS, D_MODEL), dtype=np.float32),
        "attn_weights": rng.random((4, N_TOKENS, N_TOKENS), dtype=np.float32),
        "Wg": rng.standard_normal((D_MODEL, N_EXPERT), dtype=np.float32) / 32,
        "bg": np.zeros((N_EXPERT,), np.float32),
        "W1": rng.standard_normal((N_EXPERT, D_MODEL, D_HIDDEN), dtype=np.float32) / 32,
        "b1": np.zeros((N_EXPERT, D_HIDDEN), np.float32),
        "W2": rng.standard_normal((N_EXPERT, D_HIDDEN, D_MODEL), dtype=np.float32) / 64,
        "b2": np.zeros((N_EXPERT, D_MODEL), np.float32),
    }
    o = kernel(**demo)
    print(o.shape, o.dtype)
